# revision 14
# baseline (speedup 1.0000x reference)
"""Causal self-attention (RoPE) Trainium2 Bass kernel.

Problem: B=4, T=2048, C=1024, H=16 heads, D=64, fp32 I/O.
Sharding: 8 cores = 4 (batch) x 2 (head-group TP). Each core computes
qkv/attention/proj for 1 batch and 8 heads, producing a partial
projection output; the host sums the two TP partials per batch.

Per-core pipeline (all chunk-interleaved to overlap PE matmuls with the
ACT-engine exp of the softmax):
  stage A (per 512-token chunk c): qkv projection + RoPE -> q_rot(c)
    (transient), k_rot[:, c] (persistent), v(c) (persistent, with an
    appended ones column for the softmax denominator)
  stage B (per chunk c): for each head, S_T = k^T-layout scores via
    row-packed matmuls, exp on ACT (scale=1/8), causal diag masks on
    DVE, then out_T = v_ext^T @ P_T accumulated over key blocks; row 64
    of the accumulator is the softmax denominator l. Normalize with
    reciprocal + gpsimd partition_broadcast.
  proj (per chunk): y[chunk] = o_T^T @ WprojT, partial over this core's
    512 input features.
"""

import numpy as np
from contextlib import ExitStack

import concourse.bacc as bacc
import concourse.bass as bass
import concourse.mybir as mybir
import concourse.tile as tile

# ---------------- constants ----------------
B = 4
T = 2048
C = 1024
H = 16
D = 64
L = 8  # local heads per core
NCORES = 8
ROPE_BASE = 10000.0

CH = 512  # t-chunk size
NCH = T // CH  # 4 chunks
KT = C // 128  # 8 contraction tiles
NP = L // 2  # 4 head-pair tiles
SCALE = 1.0 / np.sqrt(D)

F32 = mybir.dt.float32
BF16 = mybir.dt.bfloat16

# matmul operand dtypes
DT_X = BF16  # x / Wqkv / Wv operands
DT_K = BF16  # q_rot / k_rot
DT_PV = BF16  # P tiles, v tiles, masks
DT_O = BF16  # o_T tiles / WprojT


def _np_dt(dt):
    return mybir.dt.np(dt)


# ---------------- device kernel ----------------


def attn_body(ctx: ExitStack, tc: tile.TileContext, outs, ins):
    """outs = (y [T, C] f32,); ins = (xt4, wqk, wv, wp, cs4, sn4, mask)."""
    nc = tc.nc
    (y,) = outs if isinstance(outs, (tuple, list)) else (outs,)
    xt4, wqk, wv, wp, cs4, sn4, mask = ins

    TB = T // 128  # 16 key blocks

    consts = ctx.enter_context(tc.tile_pool(name="consts", bufs=1))
    xpool = ctx.enter_context(tc.tile_pool(name="xpool", bufs=16))
    cspool = ctx.enter_context(tc.tile_pool(name="cspool", bufs=4))
    qrpool = ctx.enter_context(tc.tile_pool(name="qrpool", bufs=8))
    rtmp = ctx.enter_context(tc.tile_pool(name="rtmp", bufs=3))
    ptpool = ctx.enter_context(tc.tile_pool(name="ptpool", bufs=4))
    otpool = ctx.enter_context(tc.tile_pool(name="otpool", bufs=8))
    yepool = ctx.enter_context(tc.tile_pool(name="yepool", bufs=3))
    lpool = ctx.enter_context(tc.tile_pool(name="lpool", bufs=3))
    pmisc = ctx.enter_context(tc.tile_pool(name="pmisc", bufs=2, space="PSUM"))
    pss_pool = ctx.enter_context(tc.tile_pool(name="pss", bufs=2, space="PSUM"))
    pso_pool = ctx.enter_context(tc.tile_pool(name="pso", bufs=2, space="PSUM"))

    # persistent tiles
    wqk_sb = [consts.tile([128, 2 * 512], DT_X, name=f"wqk{k}") for k in range(KT)]
    wv_sb = [consts.tile([128, 512], DT_X, name=f"wv{k}") for k in range(KT)]
    wp_sb = [consts.tile([128, C], DT_O, name=f"wp{p}") for p in range(NP)]
    mask_sb = consts.tile([128, 4 * CH], DT_PV, name="masks")
    k_rot = [consts.tile([128, T], DT_K, name=f"krot{p}") for p in range(NP)]
    v_sb = consts.tile([128, TB, L, 65], DT_PV, name="vsb")

    for k in range(KT):
        nc.sync.dma_start(wqk_sb[k][:], wqk[k])
        nc.sync.dma_start(wv_sb[k][:], wv[k])
    for p in range(NP):
        nc.sync.dma_start(wp_sb[p][:], wp[p])
    nc.sync.dma_start(mask_sb[:], mask[:])
    # softmax-denominator ones column
    nc.vector.memset(v_sb[:, :, :, 64:65], 1.0)

    # per-chunk transient state
    xt_sb = {}
    q_rot = {}
    cs_sb = {}
    sn_sb = {}
    ot_sb = {}

    def load_chunk_inputs(c):
        us = []

        def mk_load(c):
            def f():
                cs_sb[c] = cspool.tile([128, CH], DT_K, name=f"cs{c}", tag="cs")
                sn_sb[c] = cspool.tile([128, CH], DT_K, name=f"sn{c}", tag="sn")
                nc.sync.dma_start(cs_sb[c][:], cs4[c])
                nc.sync.dma_start(sn_sb[c][:], sn4[c])
                xt_sb[c] = []
                for k in range(KT):
                    xt = xpool.tile([128, CH], DT_X, name=f"xt{c}_{k}", tag="xt")
                    nc.sync.dma_start(xt[:], xt4[c, k])
                    xt_sb[c].append(xt)

            return f

        us.append(mk_load(c))
        return us

    def stage_a_units(c):
        """12 units: 8 q/k feature tiles + 4 v t-blocks for chunk c."""
        units = []

        def mk_qk(c, jt):
            def f():
                ps = pmisc.tile([128, CH], F32, name=f"psA{c}_{jt}", tag="pA")
                for k in range(KT):
                    nc.tensor.matmul(
                        ps[:],
                        wqk_sb[k][:, jt * 128 : (jt + 1) * 128],
                        xt_sb[c][k][:],
                        start=(k == 0),
                        stop=(k == KT - 1),
                    )
                # RoPE: rot = raw*cos + swap(raw)*sin_signed, computed in
                # bf16.  PSUM evac on DVE; the cross-partition swap runs as
                # four gpsimd copies (walrus forbids shifted all-SBUF
                # TensorTensor, but plain copies are fine); bf16 muls/adds
                # on DVE get the 4x SBUF mode.
                sn = sn_sb[c]
                q_sb = rtmp.tile([128, CH], DT_K, name=f"qsb{c}_{jt}", tag="qsb")
                nc.vector.tensor_copy(q_sb[:], ps[:])
                qsw = rtmp.tile([128, CH], DT_K, name=f"qsw{c}_{jt}", tag="qsw")
                for blk in range(2):
                    b0 = blk * 64
                    nc.gpsimd.tensor_copy(
                        qsw[b0 : b0 + 32, :], q_sb[b0 + 32 : b0 + 64, :]
                    )
                    nc.gpsimd.tensor_copy(
                        qsw[b0 + 32 : b0 + 64, :], q_sb[b0 : b0 + 32, :]
                    )
                qtmp = rtmp.tile([128, CH], DT_K, name=f"qtmp{c}_{jt}", tag="qtmp")
                nc.vector.tensor_tensor(
                    out=qtmp[:], in0=qsw[:], in1=sn[:], op=mybir.AluOpType.mult
                )
                qraw = rtmp.tile([128, CH], DT_K, name=f"qraw{c}_{jt}", tag="qraw")
                nc.vector.tensor_tensor(
                    out=qraw[:], in0=q_sb[:], in1=cs_sb[c][:],
                    op=mybir.AluOpType.mult,
                )
                if jt < NP:  # q tile
                    dst = qrpool.tile([128, CH], DT_K, name=f"qrot{c}_{jt}", tag="qr")
                    q_rot[(c, jt)] = dst
                    nc.vector.tensor_tensor(
                        out=dst[:], in0=qraw[:], in1=qtmp[:], op=mybir.AluOpType.add
                    )
                else:  # k tile
                    p = jt - NP
                    nc.vector.tensor_tensor(
                        out=k_rot[p][:, c * CH : (c + 1) * CH],
                        in0=qraw[:],
                        in1=qtmp[:],
                        op=mybir.AluOpType.add,
                    )

            return f

        def mk_v(c, tbl):
            tb = c * 4 + tbl

            def f():
                ps = pmisc.tile([128, CH], F32, name=f"psV{c}_{tbl}", tag="pA")
                for k in range(KT):
                    nc.tensor.matmul(
                        ps[:],
                        xt_sb[c][k][:, tbl * 128 : (tbl + 1) * 128],
                        wv_sb[k][:],
                        start=(k == 0),
                        stop=(k == KT - 1),
                    )
                nc.vector.tensor_copy(
                    v_sb[:, tb, :, 0:64],
                    ps[:].rearrange("p (h d) -> p h d", h=L),
                )

            return f

        for jt in range(2 * NP):
            units.append(mk_qk(c, jt))
        for tbl in range(4):
            units.append(mk_v(c, tbl))
        return units

    def stage_b_units(c):
        """per chunk c: 4 head-pairs x (2c+2) key-block-pairs."""
        units = []

        def mk_unit(c, p, jp):
            def f():
                ha, hb = 2 * p, 2 * p + 1
                qt = q_rot[(c, p)]
                kt_ = k_rot[p]
                pss = {}
                for idx, (h, rb) in enumerate(((ha, 0), (hb, 64))):
                    pss[h] = pss_pool.tile(
                        [128, 2 * CH], F32, name=f"pss{c}_{p}_{jp}_{idx}", tag="pss"
                    )
                    for half in range(2):
                        jb = 2 * jp + half
                        nc.tensor.matmul(
                            pss[h][:, half * CH : (half + 1) * CH],
                            kt_[rb : rb + 64, jb * 128 : (jb + 1) * 128],
                            qt[rb : rb + 64, :],
                            start=True,
                            stop=True,
                        )
                for idx, h in enumerate((ha, hb)):
                    pt = ptpool.tile(
                        [128, 2 * CH], DT_PV, name=f"pt{c}_{p}_{jp}_{idx}", tag="pt"
                    )
                    nc.scalar.activation(
                        pt[:],
                        pss[h][:],
                        mybir.ActivationFunctionType.Exp,
                        scale=float(SCALE),
                    )
                    if jp >= 2 * c:  # diagonal pair -> causal mask
                        moff = (jp - 2 * c) * 2 * CH
                        nc.vector.tensor_tensor(
                            out=pt[:],
                            in0=pt[:],
                            in1=mask_sb[:, moff : moff + 2 * CH],
                            op=mybir.AluOpType.mult,
                        )
                    pso = pso_unit[(c, h)]
                    for half in range(2):
                        jb = 2 * jp + half
                        nc.tensor.matmul(
                            pso[:],
                            v_sb[:, jb, h, 0:65],
                            pt[:, half * CH : (half + 1) * CH],
                            start=(jp == 0 and half == 0),
                            stop=(jp == 2 * c + 1 and half == 1),
                        )

            return f

        def mk_alloc_pso(c, p):
            def f():
                for h in (2 * p, 2 * p + 1):
                    pso_unit[(c, h)] = pso_pool.tile(
                        [65, CH], F32, name=f"pso{c}_{h}", tag="pso"
                    )

            return f

        def mk_norm(c, p):
            def f():
                ot = ot_sb[(c, p)] = otpool.tile(
                    [128, CH], DT_O, name=f"ot{c}_{p}", tag="ot"
                )
                for idx, h in enumerate((2 * p, 2 * p + 1)):
                    pso = pso_unit[(c, h)]
                    lsb = lpool.tile([1, CH], F32, name=f"lsb{c}_{h}", tag="lsb")
                    nc.vector.tensor_copy(lsb[:], pso[64:65, :])
                    linv = lpool.tile([1, CH], F32, name=f"linv{c}_{h}", tag="linv")
                    nc.vector.reciprocal_approx_fast(linv[:], lsb[:])
                    lb = lpool.tile([64, CH], F32, name=f"lb{c}_{h}", tag="lb")
                    nc.gpsimd.partition_broadcast(lb[:], linv[:])
                    nc.vector.tensor_tensor(
                        out=ot[idx * 64 : (idx + 1) * 64, :],
                        in0=pso[0:64, :],
                        in1=lb[:],
                        op=mybir.AluOpType.mult,
                    )

            return f

        pso_unit = {}
        for p in range(NP):
            units.append(mk_alloc_pso(c, p))
            for jp in range(2 * c + 2):
                units.append(mk_unit(c, p, jp))
            units.append(mk_norm(c, p))
        return units

    def proj_units(c):
        units = []

        def mk_proj(c, tbl, oc):
            def f():
                ps = pmisc.tile([128, CH], F32, name=f"psY{c}_{tbl}_{oc}", tag="pA")
                for p in range(NP):
                    nc.tensor.matmul(
                        ps[:],
                        ot_sb[(c, p)][:, tbl * 128 : (tbl + 1) * 128],
                        wp_sb[p][:, oc * CH : (oc + 1) * CH],
                        start=(p == 0),
                        stop=(p == NP - 1),
                    )
                ye = yepool.tile([128, CH], F32, name=f"ye{c}_{tbl}_{oc}", tag="ye")
                nc.vector.tensor_copy(ye[:], ps[:])
                nc.sync.dma_start(
                    y[c * CH + tbl * 128 : c * CH + (tbl + 1) * 128,
                      oc * CH : (oc + 1) * CH],
                    ye[:],
                )

            return f

        for tbl in range(4):
            for oc in range(C // CH):
                units.append(mk_proj(c, tbl, oc))
        return units

    def emit_interleaved(primary, secondary):
        """Emit primary units with secondary units spread between them."""
        if not primary:
            for u in secondary:
                u()
            return
        ns, npri = len(secondary), len(primary)
        si = 0
        for i, u in enumerate(primary):
            u()
            want = (i + 1) * ns // npri
            while si < want:
                secondary[si]()
                si += 1

    # ---- emission ----
    for u in load_chunk_inputs(0):
        u()
    for u in stage_a_units(0):
        u()
    for c in range(NCH):
        fill = []
        if c + 1 < NCH:
            fill += load_chunk_inputs(c + 1)
            fill += stage_a_units(c + 1)
        if c > 0:
            fill += proj_units(c - 1)
        emit_interleaved(stage_b_units(c), fill)
    for u in proj_units(NCH - 1):
        u()


def build_nc():
    nc = bacc.Bacc("TRN2", target_bir_lowering=False, debug=False)
    xt4 = nc.declare_dram_parameter("xt4", [NCH, KT, 128, CH], DT_X, isOutput=False)
    wqk = nc.declare_dram_parameter("wqk", [KT, 128, 1024], DT_X, isOutput=False)
    wv = nc.declare_dram_parameter("wv", [KT, 128, 512], DT_X, isOutput=False)
    wp = nc.declare_dram_parameter("wp", [NP, 128, C], DT_O, isOutput=False)
    cs4 = nc.declare_dram_parameter("cs4", [NCH, 128, CH], DT_K, isOutput=False)
    sn4 = nc.declare_dram_parameter("sn4", [NCH, 128, CH], DT_K, isOutput=False)
    mask = nc.declare_dram_parameter("mask", [128, 4 * CH], DT_PV, isOutput=False)
    yout = nc.declare_dram_parameter("y", [T, C], F32, isOutput=True)

    with tile.TileContext(nc) as tc:
        with ExitStack() as ctx:
            attn_body(
                ctx, tc, (yout[:],),
                (xt4[:], wqk[:], wv[:], wp[:], cs4[:], sn4[:], mask[:]),
            )
    nc.compile()
    return nc


# ---------------- host side ----------------


def _rope_tables_np():
    inv_freq = 1.0 / (ROPE_BASE ** (np.arange(0, D, 2, dtype=np.float64) / D))
    t = np.arange(T, dtype=np.float64)
    freqs = np.outer(t, inv_freq)  # [T, 32]
    emb = np.concatenate([freqs, freqs], axis=-1)  # [T, 64]
    return np.cos(emb), np.sin(emb)  # [T, 64] each


def _host_tables():
    cos, sin = _rope_tables_np()  # [T, 64]
    d_of_r = np.arange(128) % 64
    cs = cos[:, d_of_r].T.astype(np.float32)  # [128, T]
    sn_abs = sin[:, d_of_r].T
    sign = np.where((d_of_r % 64) < 32, -1.0, 1.0)[:, None]
    sn = (sn_abs * sign).astype(np.float32)  # [128, T]
    np_k = _np_dt(DT_K)
    cs4 = np.ascontiguousarray(cs.reshape(128, NCH, CH).transpose(1, 0, 2)).astype(np_k)
    sn4 = np.ascontiguousarray(sn.reshape(128, NCH, CH).transpose(1, 0, 2)).astype(np_k)

    jj = np.arange(128)[:, None]
    ii = np.arange(CH)[None, :]
    mask = np.zeros((128, 4 * CH), dtype=np.float64)
    for b in range(4):
        mask[:, b * CH : (b + 1) * CH] = (128 * b + jj) <= ii
    return cs4, sn4, mask


def make_core_inputs(x, Wqkv, Wproj, core):
    """Build the per-core input map (numpy arrays, device dtypes)."""
    b, g = core // 2, core % 2
    np_x = _np_dt(DT_X)
    np_pv = _np_dt(DT_PV)
    np_o = _np_dt(DT_O)

    xT = np.ascontiguousarray(x[b].T)  # [C, T]
    xt4 = np.ascontiguousarray(
        xT.reshape(KT, 128, NCH, CH).transpose(2, 0, 1, 3)
    ).astype(np_x)

    Wq = Wqkv[g * 512 : (g + 1) * 512]
    Wk = Wqkv[C + g * 512 : C + (g + 1) * 512]
    Wv = Wqkv[2 * C + g * 512 : 2 * C + (g + 1) * 512]
    wqkT = np.vstack([Wq, Wk]).T  # [C, 1024]
    wqk = np.ascontiguousarray(wqkT.reshape(KT, 128, 1024)).astype(np_x)
    wvT = Wv.T  # [C, 512]
    wv = np.ascontiguousarray(wvT.reshape(KT, 128, 512)).astype(np_x)
    wpT = Wproj[:, g * 512 : (g + 1) * 512].T  # [512, C]
    wp = np.ascontiguousarray(wpT.reshape(NP, 128, C)).astype(np_o)

    cs4, sn4, mask = _host_tables()
    return {
        "xt4": xt4,
        "wqk": wqk,
        "wv": wv,
        "wp": wp,
        "cs4": cs4,
        "sn4": sn4,
        "mask": mask.astype(np_pv),
    }


LAST_RESULTS = None


def kernel(x, Wqkv, Wproj):
    global LAST_RESULTS
    from concourse.bass_utils import run_bass_kernel_spmd

    x = np.asarray(x, dtype=np.float32)
    Wqkv = np.asarray(Wqkv, dtype=np.float32)
    Wproj = np.asarray(Wproj, dtype=np.float32)

    nc = build_nc()
    in_maps = [make_core_inputs(x, Wqkv, Wproj, core) for core in range(NCORES)]
    res = run_bass_kernel_spmd(nc, in_maps, list(range(NCORES)))
    LAST_RESULTS = res

    out = np.empty((B, T, C), dtype=np.float32)
    for b in range(B):
        out[b] = res.results[2 * b]["y"] + res.results[2 * b + 1]["y"]
    return out


# revision 17
# speedup vs baseline: 1.5130x; 1.5130x over previous
"""Causal self-attention (RoPE) Trainium2 Bass kernel.

Problem: B=4, T=2048, C=1024, H=16 heads, D=64, fp32 I/O.
Sharding: 8 cores = 4 (batch) x 2 (head-group TP). Each core computes
qkv/attention/proj for 1 batch and 8 heads, producing a partial
projection output; the host sums the two TP partials per batch.

Per-core pipeline (all chunk-interleaved to overlap PE matmuls with the
ACT-engine exp of the softmax):
  stage A (per 512-token chunk c): qkv projection + RoPE -> q_rot(c)
    (transient), k_rot[:, c] (persistent), v(c) (persistent, with an
    appended ones column for the softmax denominator)
  stage B (per chunk c): for each head, S_T = k^T-layout scores via
    row-packed matmuls, exp on ACT (scale=1/8), causal diag masks on
    DVE, then out_T = v_ext^T @ P_T accumulated over key blocks; row 64
    of the accumulator is the softmax denominator l. Normalize with
    reciprocal + gpsimd partition_broadcast.
  proj (per chunk): y[chunk] = o_T^T @ WprojT, partial over this core's
    512 input features.
"""

import numpy as np
from contextlib import ExitStack

import concourse.bacc as bacc
import concourse.bass as bass
import concourse.mybir as mybir
import concourse.tile as tile

# ---------------- constants ----------------
B = 4
T = 2048
C = 1024
H = 16
D = 64
L = 8  # local heads per core
NCORES = 8
ROPE_BASE = 10000.0

CH = 512  # t-chunk size
NCH = T // CH  # 4 chunks
KT = C // 128  # 8 contraction tiles
NP = L // 2  # 4 head-pair tiles
SCALE = 1.0 / np.sqrt(D)

F32 = mybir.dt.float32
BF16 = mybir.dt.bfloat16

# matmul operand dtypes
DT_X = BF16  # x / Wqkv / Wv operands
DT_K = BF16  # q_rot / k_rot
DT_PV = BF16  # P tiles, v tiles, masks
DT_O = BF16  # o_T tiles / WprojT


def _np_dt(dt):
    return mybir.dt.np(dt)


# ---------------- device kernel ----------------


def attn_body(ctx: ExitStack, tc: tile.TileContext, outs, ins):
    """outs = (y [T, C] f32,); ins = (xt4, wqk, wv, wp, cs4, sn4, mask)."""
    nc = tc.nc
    (y,) = outs if isinstance(outs, (tuple, list)) else (outs,)
    xt4, wqk, wv, wp, cs4, sn4, mask = ins

    TB = T // 128  # 16 key blocks

    consts = ctx.enter_context(tc.tile_pool(name="consts", bufs=1))
    xpool = ctx.enter_context(tc.tile_pool(name="xpool", bufs=16))
    cspool = ctx.enter_context(tc.tile_pool(name="cspool", bufs=4))
    qrpool = ctx.enter_context(tc.tile_pool(name="qrpool", bufs=8))
    rtmp = ctx.enter_context(tc.tile_pool(name="rtmp", bufs=3))
    ptpool = ctx.enter_context(tc.tile_pool(name="ptpool", bufs=4))
    otpool = ctx.enter_context(tc.tile_pool(name="otpool", bufs=8))
    yepool = ctx.enter_context(tc.tile_pool(name="yepool", bufs=3))
    lpool = ctx.enter_context(tc.tile_pool(name="lpool", bufs=3))
    pmisc = ctx.enter_context(tc.tile_pool(name="pmisc", bufs=2, space="PSUM"))
    pss_pool = ctx.enter_context(tc.tile_pool(name="pss", bufs=2, space="PSUM"))
    pso_pool = ctx.enter_context(tc.tile_pool(name="pso", bufs=2, space="PSUM"))

    # persistent tiles
    wqk_sb = [consts.tile([128, 2 * 512], DT_X, name=f"wqk{k}") for k in range(KT)]
    wv_sb = [consts.tile([128, 512], DT_X, name=f"wv{k}") for k in range(KT)]
    wp_sb = [consts.tile([128, C], DT_O, name=f"wp{p}") for p in range(NP)]
    mask_sb = consts.tile([128, 4 * CH], DT_PV, name="masks")
    k_rot = [consts.tile([128, T], DT_K, name=f"krot{p}") for p in range(NP)]
    v_sb = consts.tile([128, TB, L, 65], DT_PV, name="vsb")

    def load_consts_early():
        for k in range(KT):
            nc.sync.dma_start(wqk_sb[k][:], wqk[k])
            nc.sync.dma_start(wv_sb[k][:], wv[k])
        # softmax-denominator ones column
        nc.vector.memset(v_sb[:, :, :, 64:65], 1.0)

    def load_consts_late():
        nc.sync.dma_start(mask_sb[:], mask[:])
        for p in range(NP):
            nc.sync.dma_start(wp_sb[p][:], wp[p])

    # per-chunk transient state
    xt_sb = {}
    q_rot = {}
    cs_sb = {}
    sn_sb = {}
    ot_sb = {}

    def load_chunk_inputs(c):
        us = []

        def mk_load(c):
            def f():
                cs_sb[c] = cspool.tile([128, CH], DT_K, name=f"cs{c}", tag="cs")
                sn_sb[c] = cspool.tile([128, CH], DT_K, name=f"sn{c}", tag="sn")
                nc.sync.dma_start(cs_sb[c][:], cs4[c])
                nc.sync.dma_start(sn_sb[c][:], sn4[c])
                xt_sb[c] = []
                for k in range(KT):
                    xt = xpool.tile([128, CH], DT_X, name=f"xt{c}_{k}", tag="xt")
                    nc.sync.dma_start(xt[:], xt4[c, k])
                    xt_sb[c].append(xt)

            return f

        us.append(mk_load(c))
        return us

    def stage_a_units(c):
        """12 units: 8 q/k feature tiles + 4 v t-blocks for chunk c."""
        units = []

        def mk_qk(c, jt):
            def f():
                ps = pmisc.tile([128, CH], F32, name=f"psA{c}_{jt}", tag="pA")
                for k in range(KT):
                    nc.tensor.matmul(
                        ps[:],
                        wqk_sb[k][:, jt * 128 : (jt + 1) * 128],
                        xt_sb[c][k][:],
                        start=(k == 0),
                        stop=(k == KT - 1),
                    )
                # RoPE: rot = raw*cos + swap(raw)*sin_signed, computed in
                # bf16.  PSUM evac on DVE; the cross-partition swap runs as
                # four gpsimd copies (walrus forbids shifted all-SBUF
                # TensorTensor, but plain copies are fine); bf16 muls/adds
                # on DVE get the 4x SBUF mode.
                sn = sn_sb[c]
                q_sb = rtmp.tile([128, CH], DT_K, name=f"qsb{c}_{jt}", tag="qsb")
                nc.vector.tensor_copy(q_sb[:], ps[:])
                qsw = rtmp.tile([128, CH], DT_K, name=f"qsw{c}_{jt}", tag="qsw")
                for blk in range(2):
                    b0 = blk * 64
                    nc.vector.tensor_copy(
                        qsw[b0 : b0 + 32, :], q_sb[b0 + 32 : b0 + 64, :]
                    )
                    nc.vector.tensor_copy(
                        qsw[b0 + 32 : b0 + 64, :], q_sb[b0 : b0 + 32, :]
                    )
                qtmp = rtmp.tile([128, CH], DT_K, name=f"qtmp{c}_{jt}", tag="qtmp")
                nc.vector.tensor_tensor(
                    out=qtmp[:], in0=qsw[:], in1=sn[:], op=mybir.AluOpType.mult
                )
                qraw = rtmp.tile([128, CH], DT_K, name=f"qraw{c}_{jt}", tag="qraw")
                nc.vector.tensor_tensor(
                    out=qraw[:], in0=q_sb[:], in1=cs_sb[c][:],
                    op=mybir.AluOpType.mult,
                )
                if jt < NP:  # q tile
                    dst = qrpool.tile([128, CH], DT_K, name=f"qrot{c}_{jt}", tag="qr")
                    q_rot[(c, jt)] = dst
                    nc.vector.tensor_tensor(
                        out=dst[:], in0=qraw[:], in1=qtmp[:], op=mybir.AluOpType.add
                    )
                else:  # k tile
                    p = jt - NP
                    nc.vector.tensor_tensor(
                        out=k_rot[p][:, c * CH : (c + 1) * CH],
                        in0=qraw[:],
                        in1=qtmp[:],
                        op=mybir.AluOpType.add,
                    )

            return f

        def mk_v(c, tbl):
            tb = c * 4 + tbl

            def f():
                ps = pmisc.tile([128, CH], F32, name=f"psV{c}_{tbl}", tag="pA")
                for k in range(KT):
                    nc.tensor.matmul(
                        ps[:],
                        xt_sb[c][k][:, tbl * 128 : (tbl + 1) * 128],
                        wv_sb[k][:],
                        start=(k == 0),
                        stop=(k == KT - 1),
                    )
                nc.vector.tensor_copy(
                    v_sb[:, tb, :, 0:64],
                    ps[:].rearrange("p (h d) -> p h d", h=L),
                )

            return f

        for jt in range(2 * NP):
            units.append(mk_qk(c, jt))
        for tbl in range(4):
            units.append(mk_v(c, tbl))
        return units

    def stage_b_units(c):
        """per chunk c: 4 head-pairs x (2c+2) key-block-pairs."""
        units = []

        def mk_unit(c, p, jp):
            def f():
                ha, hb = 2 * p, 2 * p + 1
                qt = q_rot[(c, p)]
                kt_ = k_rot[p]
                pss = {}
                for idx, (h, rb) in enumerate(((ha, 0), (hb, 64))):
                    pss[h] = pss_pool.tile(
                        [128, 2 * CH], F32, name=f"pss{c}_{p}_{jp}_{idx}", tag="pss"
                    )
                    for half in range(2):
                        jb = 2 * jp + half
                        nc.tensor.matmul(
                            pss[h][:, half * CH : (half + 1) * CH],
                            kt_[rb : rb + 64, jb * 128 : (jb + 1) * 128],
                            qt[rb : rb + 64, :],
                            start=True,
                            stop=True,
                        )
                for idx, h in enumerate((ha, hb)):
                    pt = ptpool.tile(
                        [128, 2 * CH], DT_PV, name=f"pt{c}_{p}_{jp}_{idx}", tag="pt"
                    )
                    nc.scalar.activation(
                        pt[:],
                        pss[h][:],
                        mybir.ActivationFunctionType.Exp,
                        scale=float(SCALE),
                    )
                    if jp >= 2 * c:  # diagonal pair -> causal mask
                        moff = (jp - 2 * c) * 2 * CH
                        nc.vector.tensor_tensor(
                            out=pt[:],
                            in0=pt[:],
                            in1=mask_sb[:, moff : moff + 2 * CH],
                            op=mybir.AluOpType.mult,
                        )
                    pso = pso_unit[(c, h)]
                    for half in range(2):
                        jb = 2 * jp + half
                        nc.tensor.matmul(
                            pso[:],
                            v_sb[:, jb, h, 0:65],
                            pt[:, half * CH : (half + 1) * CH],
                            start=(jp == 0 and half == 0),
                            stop=(jp == 2 * c + 1 and half == 1),
                        )

            return f

        def mk_alloc_pso(c, p):
            def f():
                for h in (2 * p, 2 * p + 1):
                    pso_unit[(c, h)] = pso_pool.tile(
                        [65, CH], F32, name=f"pso{c}_{h}", tag="pso"
                    )

            return f

        def mk_norm(c, p):
            def f():
                ot = ot_sb[(c, p)] = otpool.tile(
                    [128, CH], DT_O, name=f"ot{c}_{p}", tag="ot"
                )
                for idx, h in enumerate((2 * p, 2 * p + 1)):
                    pso = pso_unit[(c, h)]
                    lsb = lpool.tile([1, CH], F32, name=f"lsb{c}_{h}", tag="lsb")
                    nc.vector.tensor_copy(lsb[:], pso[64:65, :])
                    linv = lpool.tile([1, CH], F32, name=f"linv{c}_{h}", tag="linv")
                    nc.vector.reciprocal_approx_fast(linv[:], lsb[:])
                    lb = lpool.tile([64, CH], F32, name=f"lb{c}_{h}", tag="lb")
                    nc.gpsimd.partition_broadcast(lb[:], linv[:])
                    nc.vector.tensor_tensor(
                        out=ot[idx * 64 : (idx + 1) * 64, :],
                        in0=pso[0:64, :],
                        in1=lb[:],
                        op=mybir.AluOpType.mult,
                    )

            return f

        pso_unit = {}
        for p in range(NP):
            units.append(mk_alloc_pso(c, p))
            for jp in range(2 * c + 2):
                units.append(mk_unit(c, p, jp))
            units.append(mk_norm(c, p))
        return units

    def proj_units(c):
        units = []

        def mk_proj(c, tbl, oc):
            def f():
                ps = pmisc.tile([128, CH], F32, name=f"psY{c}_{tbl}_{oc}", tag="pA")
                for p in range(NP):
                    nc.tensor.matmul(
                        ps[:],
                        ot_sb[(c, p)][:, tbl * 128 : (tbl + 1) * 128],
                        wp_sb[p][:, oc * CH : (oc + 1) * CH],
                        start=(p == 0),
                        stop=(p == NP - 1),
                    )
                ye = yepool.tile([128, CH], F32, name=f"ye{c}_{tbl}_{oc}", tag="ye")
                nc.vector.tensor_copy(ye[:], ps[:])
                nc.sync.dma_start(
                    y[c * CH + tbl * 128 : c * CH + (tbl + 1) * 128,
                      oc * CH : (oc + 1) * CH],
                    ye[:],
                )

            return f

        for tbl in range(4):
            for oc in range(C // CH):
                units.append(mk_proj(c, tbl, oc))
        return units

    def emit_interleaved(primary, secondary):
        """Emit primary units with secondary units spread between them."""
        if not primary:
            for u in secondary:
                u()
            return
        ns, npri = len(secondary), len(primary)
        si = 0
        for i, u in enumerate(primary):
            u()
            want = (i + 1) * ns // npri
            while si < want:
                secondary[si]()
                si += 1

    # ---- emission ----
    for u in load_chunk_inputs(0):
        u()
    load_consts_early()
    for u in stage_a_units(0):
        u()
    load_consts_late()
    for c in range(NCH):
        fill = []
        if c + 1 < NCH:
            fill += load_chunk_inputs(c + 1)
            fill += stage_a_units(c + 1)
        if c > 0:
            fill += proj_units(c - 1)
        emit_interleaved(stage_b_units(c), fill)
    for u in proj_units(NCH - 1):
        u()


def build_nc():
    nc = bacc.Bacc("TRN2", target_bir_lowering=False, debug=False)
    xt4 = nc.declare_dram_parameter("xt4", [NCH, KT, 128, CH], DT_X, isOutput=False)
    wqk = nc.declare_dram_parameter("wqk", [KT, 128, 1024], DT_X, isOutput=False)
    wv = nc.declare_dram_parameter("wv", [KT, 128, 512], DT_X, isOutput=False)
    wp = nc.declare_dram_parameter("wp", [NP, 128, C], DT_O, isOutput=False)
    cs4 = nc.declare_dram_parameter("cs4", [NCH, 128, CH], DT_K, isOutput=False)
    sn4 = nc.declare_dram_parameter("sn4", [NCH, 128, CH], DT_K, isOutput=False)
    mask = nc.declare_dram_parameter("mask", [128, 4 * CH], DT_PV, isOutput=False)
    yout = nc.declare_dram_parameter("y", [T, C], F32, isOutput=True)

    with tile.TileContext(nc) as tc:
        with ExitStack() as ctx:
            attn_body(
                ctx, tc, (yout[:],),
                (xt4[:], wqk[:], wv[:], wp[:], cs4[:], sn4[:], mask[:]),
            )
    nc.compile()
    return nc


# ---------------- host side ----------------


def _rope_tables_np():
    inv_freq = 1.0 / (ROPE_BASE ** (np.arange(0, D, 2, dtype=np.float64) / D))
    t = np.arange(T, dtype=np.float64)
    freqs = np.outer(t, inv_freq)  # [T, 32]
    emb = np.concatenate([freqs, freqs], axis=-1)  # [T, 64]
    return np.cos(emb), np.sin(emb)  # [T, 64] each


def _host_tables():
    cos, sin = _rope_tables_np()  # [T, 64]
    d_of_r = np.arange(128) % 64
    cs = cos[:, d_of_r].T.astype(np.float32)  # [128, T]
    sn_abs = sin[:, d_of_r].T
    sign = np.where((d_of_r % 64) < 32, -1.0, 1.0)[:, None]
    sn = (sn_abs * sign).astype(np.float32)  # [128, T]
    np_k = _np_dt(DT_K)
    cs4 = np.ascontiguousarray(cs.reshape(128, NCH, CH).transpose(1, 0, 2)).astype(np_k)
    sn4 = np.ascontiguousarray(sn.reshape(128, NCH, CH).transpose(1, 0, 2)).astype(np_k)

    jj = np.arange(128)[:, None]
    ii = np.arange(CH)[None, :]
    mask = np.zeros((128, 4 * CH), dtype=np.float64)
    for b in range(4):
        mask[:, b * CH : (b + 1) * CH] = (128 * b + jj) <= ii
    return cs4, sn4, mask


def make_core_inputs(x, Wqkv, Wproj, core):
    """Build the per-core input map (numpy arrays, device dtypes)."""
    b, g = core // 2, core % 2
    np_x = _np_dt(DT_X)
    np_pv = _np_dt(DT_PV)
    np_o = _np_dt(DT_O)

    xT = np.ascontiguousarray(x[b].T)  # [C, T]
    xt4 = np.ascontiguousarray(
        xT.reshape(KT, 128, NCH, CH).transpose(2, 0, 1, 3)
    ).astype(np_x)

    Wq = Wqkv[g * 512 : (g + 1) * 512]
    Wk = Wqkv[C + g * 512 : C + (g + 1) * 512]
    Wv = Wqkv[2 * C + g * 512 : 2 * C + (g + 1) * 512]
    wqkT = np.vstack([Wq, Wk]).T  # [C, 1024]
    wqk = np.ascontiguousarray(wqkT.reshape(KT, 128, 1024)).astype(np_x)
    wvT = Wv.T  # [C, 512]
    wv = np.ascontiguousarray(wvT.reshape(KT, 128, 512)).astype(np_x)
    wpT = Wproj[:, g * 512 : (g + 1) * 512].T  # [512, C]
    wp = np.ascontiguousarray(wpT.reshape(NP, 128, C)).astype(np_o)

    cs4, sn4, mask = _host_tables()
    return {
        "xt4": xt4,
        "wqk": wqk,
        "wv": wv,
        "wp": wp,
        "cs4": cs4,
        "sn4": sn4,
        "mask": mask.astype(np_pv),
    }


LAST_RESULTS = None


def kernel(x, Wqkv, Wproj):
    global LAST_RESULTS
    from concourse.bass_utils import run_bass_kernel_spmd

    x = np.asarray(x, dtype=np.float32)
    Wqkv = np.asarray(Wqkv, dtype=np.float32)
    Wproj = np.asarray(Wproj, dtype=np.float32)

    nc = build_nc()
    in_maps = [make_core_inputs(x, Wqkv, Wproj, core) for core in range(NCORES)]
    res = run_bass_kernel_spmd(nc, in_maps, list(range(NCORES)))
    LAST_RESULTS = res

    out = np.empty((B, T, C), dtype=np.float32)
    for b in range(B):
        out[b] = res.results[2 * b]["y"] + res.results[2 * b + 1]["y"]
    return out


# revision 19
# speedup vs baseline: 1.5342x; 1.0140x over previous
"""Causal self-attention (RoPE) Trainium2 Bass kernel.

Problem: B=4, T=2048, C=1024, H=16 heads, D=64, fp32 I/O.
Sharding: 8 cores = 4 (batch) x 2 (head-group TP). Each core computes
qkv/attention/proj for 1 batch and 8 heads, producing a partial
projection output; the host sums the two TP partials per batch.

Per-core pipeline (all chunk-interleaved to overlap PE matmuls with the
ACT-engine exp of the softmax):
  stage A (per 512-token chunk c): qkv projection + RoPE -> q_rot(c)
    (transient), k_rot[:, c] (persistent), v(c) (persistent, with an
    appended ones column for the softmax denominator)
  stage B (per chunk c): for each head, S_T = k^T-layout scores via
    row-packed matmuls, exp on ACT (scale=1/8), causal diag masks on
    DVE, then out_T = v_ext^T @ P_T accumulated over key blocks; row 64
    of the accumulator is the softmax denominator l. Normalize with
    reciprocal + gpsimd partition_broadcast.
  proj (per chunk): y[chunk] = o_T^T @ WprojT, partial over this core's
    512 input features.
"""

import numpy as np
from contextlib import ExitStack

import concourse.bacc as bacc
import concourse.bass as bass
import concourse.mybir as mybir
import concourse.tile as tile

# ---------------- constants ----------------
B = 4
T = 2048
C = 1024
H = 16
D = 64
L = 8  # local heads per core
NCORES = 8
ROPE_BASE = 10000.0

CH = 512  # t-chunk size
NCH = T // CH  # 4 chunks
KT = C // 128  # 8 contraction tiles
NP = L // 2  # 4 head-pair tiles
SCALE = 1.0 / np.sqrt(D)

F32 = mybir.dt.float32
BF16 = mybir.dt.bfloat16

# matmul operand dtypes
DT_X = BF16  # x / Wqkv / Wv operands
DT_K = BF16  # q_rot / k_rot
DT_PV = BF16  # P tiles, v tiles, masks
DT_O = BF16  # o_T tiles / WprojT


def _np_dt(dt):
    return mybir.dt.np(dt)


# ---------------- device kernel ----------------


def attn_body(ctx: ExitStack, tc: tile.TileContext, outs, ins):
    """outs = (y [T, C] f32,); ins = (xt4, wqk, wv, wp, cs4, sn4, mask)."""
    nc = tc.nc
    (y,) = outs if isinstance(outs, (tuple, list)) else (outs,)
    xt4, wqk, wv, wp, cs4, sn4, mask = ins

    TB = T // 128  # 16 key blocks

    consts = ctx.enter_context(tc.tile_pool(name="consts", bufs=1))
    xpool = ctx.enter_context(tc.tile_pool(name="xpool", bufs=16))
    cspool = ctx.enter_context(tc.tile_pool(name="cspool", bufs=4))
    qrpool = ctx.enter_context(tc.tile_pool(name="qrpool", bufs=8))
    rtmp = ctx.enter_context(tc.tile_pool(name="rtmp", bufs=4))
    ptpool = ctx.enter_context(tc.tile_pool(name="ptpool", bufs=6))
    otpool = ctx.enter_context(tc.tile_pool(name="otpool", bufs=8))
    yepool = ctx.enter_context(tc.tile_pool(name="yepool", bufs=3))
    lpool = ctx.enter_context(tc.tile_pool(name="lpool", bufs=3))
    pmisc = ctx.enter_context(tc.tile_pool(name="pmisc", bufs=2, space="PSUM"))
    pss_pool = ctx.enter_context(tc.tile_pool(name="pss", bufs=2, space="PSUM"))
    pso_pool = ctx.enter_context(tc.tile_pool(name="pso", bufs=2, space="PSUM"))

    # persistent tiles
    wqk_sb = [consts.tile([128, 2 * 512], DT_X, name=f"wqk{k}") for k in range(KT)]
    wv_sb = [consts.tile([128, 512], DT_X, name=f"wv{k}") for k in range(KT)]
    wp_sb = [consts.tile([128, C], DT_O, name=f"wp{p}") for p in range(NP)]
    mask_sb = consts.tile([128, 4 * CH], DT_PV, name="masks")
    k_rot = [consts.tile([128, T], DT_K, name=f"krot{p}") for p in range(NP)]
    v_sb = consts.tile([128, TB, L, 65], DT_PV, name="vsb")

    def load_consts_early():
        for k in range(KT):
            nc.sync.dma_start(wqk_sb[k][:], wqk[k])
            nc.sync.dma_start(wv_sb[k][:], wv[k])
        # softmax-denominator ones column
        nc.vector.memset(v_sb[:, :, :, 64:65], 1.0)

    def load_consts_late():
        nc.sync.dma_start(mask_sb[:], mask[:])
        for p in range(NP):
            nc.sync.dma_start(wp_sb[p][:], wp[p])

    # per-chunk transient state
    xt_sb = {}
    q_rot = {}
    cs_sb = {}
    sn_sb = {}
    ot_sb = {}

    def load_chunk_inputs(c):
        us = []

        def mk_load(c):
            def f():
                cs_sb[c] = cspool.tile([128, CH], DT_K, name=f"cs{c}", tag="cs")
                sn_sb[c] = cspool.tile([128, CH], DT_K, name=f"sn{c}", tag="sn")
                nc.sync.dma_start(cs_sb[c][:], cs4[c])
                nc.sync.dma_start(sn_sb[c][:], sn4[c])
                xt_sb[c] = []
                for k in range(KT):
                    xt = xpool.tile([128, CH], DT_X, name=f"xt{c}_{k}", tag="xt")
                    nc.sync.dma_start(xt[:], xt4[c, k])
                    xt_sb[c].append(xt)

            return f

        us.append(mk_load(c))
        return us

    def stage_a_units(c):
        """12 units: 8 q/k feature tiles + 4 v t-blocks for chunk c."""
        units = []

        def mk_qk(c, jt):
            def f():
                ps = pmisc.tile([128, CH], F32, name=f"psA{c}_{jt}", tag="pA")
                for k in range(KT):
                    nc.tensor.matmul(
                        ps[:],
                        wqk_sb[k][:, jt * 128 : (jt + 1) * 128],
                        xt_sb[c][k][:],
                        start=(k == 0),
                        stop=(k == KT - 1),
                    )
                # RoPE: rot = raw*cos + swap(raw)*sin_signed, computed in
                # bf16.  PSUM evac on DVE; the cross-partition swap runs as
                # four gpsimd copies (walrus forbids shifted all-SBUF
                # TensorTensor, but plain copies are fine); bf16 muls/adds
                # on DVE get the 4x SBUF mode.
                sn = sn_sb[c]
                q_sb = rtmp.tile([128, CH], DT_K, name=f"qsb{c}_{jt}", tag="qsb")
                nc.vector.tensor_copy(q_sb[:], ps[:])
                qsw = rtmp.tile([128, CH], DT_K, name=f"qsw{c}_{jt}", tag="qsw")
                for blk in range(2):
                    b0 = blk * 64
                    nc.vector.tensor_copy(
                        qsw[b0 : b0 + 32, :], q_sb[b0 + 32 : b0 + 64, :]
                    )
                    nc.vector.tensor_copy(
                        qsw[b0 + 32 : b0 + 64, :], q_sb[b0 : b0 + 32, :]
                    )
                qtmp = rtmp.tile([128, CH], DT_K, name=f"qtmp{c}_{jt}", tag="qtmp")
                nc.vector.tensor_tensor(
                    out=qtmp[:], in0=qsw[:], in1=sn[:], op=mybir.AluOpType.mult
                )
                qraw = rtmp.tile([128, CH], DT_K, name=f"qraw{c}_{jt}", tag="qraw")
                nc.vector.tensor_tensor(
                    out=qraw[:], in0=q_sb[:], in1=cs_sb[c][:],
                    op=mybir.AluOpType.mult,
                )
                if jt < NP:  # q tile
                    dst = qrpool.tile([128, CH], DT_K, name=f"qrot{c}_{jt}", tag="qr")
                    q_rot[(c, jt)] = dst
                    nc.vector.tensor_tensor(
                        out=dst[:], in0=qraw[:], in1=qtmp[:], op=mybir.AluOpType.add
                    )
                else:  # k tile
                    p = jt - NP
                    nc.vector.tensor_tensor(
                        out=k_rot[p][:, c * CH : (c + 1) * CH],
                        in0=qraw[:],
                        in1=qtmp[:],
                        op=mybir.AluOpType.add,
                    )

            return f

        def mk_v(c, tbl):
            tb = c * 4 + tbl

            def f():
                ps = pmisc.tile([128, CH], F32, name=f"psV{c}_{tbl}", tag="pA")
                for k in range(KT):
                    nc.tensor.matmul(
                        ps[:],
                        xt_sb[c][k][:, tbl * 128 : (tbl + 1) * 128],
                        wv_sb[k][:],
                        start=(k == 0),
                        stop=(k == KT - 1),
                    )
                nc.vector.tensor_copy(
                    v_sb[:, tb, :, 0:64],
                    ps[:].rearrange("p (h d) -> p h d", h=L),
                )

            return f

        for jt in range(2 * NP):
            units.append(mk_qk(c, jt))
        for tbl in range(4):
            units.append(mk_v(c, tbl))
        return units

    def stage_b_units(c):
        """per chunk c: 4 head-pairs x (2c+2) key-block-pairs."""
        units = []

        def mk_unit(c, p, jp):
            def f():
                ha, hb = 2 * p, 2 * p + 1
                qt = q_rot[(c, p)]
                kt_ = k_rot[p]
                pss = {}
                for idx, (h, rb) in enumerate(((ha, 0), (hb, 64))):
                    pss[h] = pss_pool.tile(
                        [128, 2 * CH], F32, name=f"pss{c}_{p}_{jp}_{idx}", tag="pss"
                    )
                    for half in range(2):
                        jb = 2 * jp + half
                        nc.tensor.matmul(
                            pss[h][:, half * CH : (half + 1) * CH],
                            kt_[rb : rb + 64, jb * 128 : (jb + 1) * 128],
                            qt[rb : rb + 64, :],
                            start=True,
                            stop=True,
                        )
                for idx, h in enumerate((ha, hb)):
                    pt = ptpool.tile(
                        [128, 2 * CH], DT_PV, name=f"pt{c}_{p}_{jp}_{idx}", tag="pt"
                    )
                    nc.scalar.activation(
                        pt[:],
                        pss[h][:],
                        mybir.ActivationFunctionType.Exp,
                        scale=float(SCALE),
                    )
                    if jp >= 2 * c:  # diagonal pair -> causal mask
                        moff = (jp - 2 * c) * 2 * CH
                        nc.vector.tensor_tensor(
                            out=pt[:],
                            in0=pt[:],
                            in1=mask_sb[:, moff : moff + 2 * CH],
                            op=mybir.AluOpType.mult,
                        )
                    pso = pso_unit[(c, h)]
                    for half in range(2):
                        jb = 2 * jp + half
                        nc.tensor.matmul(
                            pso[:],
                            v_sb[:, jb, h, 0:65],
                            pt[:, half * CH : (half + 1) * CH],
                            start=(jp == 0 and half == 0),
                            stop=(jp == 2 * c + 1 and half == 1),
                        )

            return f

        def mk_alloc_pso(c, p):
            def f():
                for h in (2 * p, 2 * p + 1):
                    pso_unit[(c, h)] = pso_pool.tile(
                        [65, CH], F32, name=f"pso{c}_{h}", tag="pso"
                    )

            return f

        def mk_norm(c, p):
            def f():
                ot = ot_sb[(c, p)] = otpool.tile(
                    [128, CH], DT_O, name=f"ot{c}_{p}", tag="ot"
                )
                for idx, h in enumerate((2 * p, 2 * p + 1)):
                    pso = pso_unit[(c, h)]
                    lsb = lpool.tile([1, CH], F32, name=f"lsb{c}_{h}", tag="lsb")
                    nc.vector.tensor_copy(lsb[:], pso[64:65, :])
                    linv = lpool.tile([1, CH], F32, name=f"linv{c}_{h}", tag="linv")
                    nc.vector.reciprocal_approx_fast(linv[:], lsb[:])
                    lb = lpool.tile([64, CH], F32, name=f"lb{c}_{h}", tag="lb")
                    nc.gpsimd.partition_broadcast(lb[:], linv[:])
                    nc.vector.tensor_tensor(
                        out=ot[idx * 64 : (idx + 1) * 64, :],
                        in0=pso[0:64, :],
                        in1=lb[:],
                        op=mybir.AluOpType.mult,
                    )

            return f

        pso_unit = {}
        for p in range(NP):
            units.append(mk_alloc_pso(c, p))
            for jp in range(2 * c + 2):
                units.append(mk_unit(c, p, jp))
            units.append(mk_norm(c, p))
        return units

    def proj_units(c):
        units = []

        def mk_proj(c, tbl, oc):
            def f():
                ps = pmisc.tile([128, CH], F32, name=f"psY{c}_{tbl}_{oc}", tag="pA")
                for p in range(NP):
                    nc.tensor.matmul(
                        ps[:],
                        ot_sb[(c, p)][:, tbl * 128 : (tbl + 1) * 128],
                        wp_sb[p][:, oc * CH : (oc + 1) * CH],
                        start=(p == 0),
                        stop=(p == NP - 1),
                    )
                ye = yepool.tile([128, CH], F32, name=f"ye{c}_{tbl}_{oc}", tag="ye")
                nc.vector.tensor_copy(ye[:], ps[:])
                nc.sync.dma_start(
                    y[c * CH + tbl * 128 : c * CH + (tbl + 1) * 128,
                      oc * CH : (oc + 1) * CH],
                    ye[:],
                )

            return f

        for tbl in range(4):
            for oc in range(C // CH):
                units.append(mk_proj(c, tbl, oc))
        return units

    def emit_interleaved(primary, secondary):
        """Emit primary units with secondary units spread between them."""
        if not primary:
            for u in secondary:
                u()
            return
        ns, npri = len(secondary), len(primary)
        si = 0
        for i, u in enumerate(primary):
            u()
            want = (i + 1) * ns // npri
            while si < want:
                secondary[si]()
                si += 1

    # ---- emission ----
    for u in load_chunk_inputs(0):
        u()
    load_consts_early()
    for u in stage_a_units(0):
        u()
    load_consts_late()
    for c in range(NCH):
        fill = []
        if c + 1 < NCH:
            fill += load_chunk_inputs(c + 1)
            fill += stage_a_units(c + 1)
        if c > 0:
            fill += proj_units(c - 1)
        emit_interleaved(stage_b_units(c), fill)
    for u in proj_units(NCH - 1):
        u()


def build_nc():
    nc = bacc.Bacc("TRN2", target_bir_lowering=False, debug=False)
    xt4 = nc.declare_dram_parameter("xt4", [NCH, KT, 128, CH], DT_X, isOutput=False)
    wqk = nc.declare_dram_parameter("wqk", [KT, 128, 1024], DT_X, isOutput=False)
    wv = nc.declare_dram_parameter("wv", [KT, 128, 512], DT_X, isOutput=False)
    wp = nc.declare_dram_parameter("wp", [NP, 128, C], DT_O, isOutput=False)
    cs4 = nc.declare_dram_parameter("cs4", [NCH, 128, CH], DT_K, isOutput=False)
    sn4 = nc.declare_dram_parameter("sn4", [NCH, 128, CH], DT_K, isOutput=False)
    mask = nc.declare_dram_parameter("mask", [128, 4 * CH], DT_PV, isOutput=False)
    yout = nc.declare_dram_parameter("y", [T, C], F32, isOutput=True)

    with tile.TileContext(nc) as tc:
        with ExitStack() as ctx:
            attn_body(
                ctx, tc, (yout[:],),
                (xt4[:], wqk[:], wv[:], wp[:], cs4[:], sn4[:], mask[:]),
            )
    nc.compile()
    return nc


# ---------------- host side ----------------


def _rope_tables_np():
    inv_freq = 1.0 / (ROPE_BASE ** (np.arange(0, D, 2, dtype=np.float64) / D))
    t = np.arange(T, dtype=np.float64)
    freqs = np.outer(t, inv_freq)  # [T, 32]
    emb = np.concatenate([freqs, freqs], axis=-1)  # [T, 64]
    return np.cos(emb), np.sin(emb)  # [T, 64] each


def _host_tables():
    cos, sin = _rope_tables_np()  # [T, 64]
    d_of_r = np.arange(128) % 64
    cs = cos[:, d_of_r].T.astype(np.float32)  # [128, T]
    sn_abs = sin[:, d_of_r].T
    sign = np.where((d_of_r % 64) < 32, -1.0, 1.0)[:, None]
    sn = (sn_abs * sign).astype(np.float32)  # [128, T]
    np_k = _np_dt(DT_K)
    cs4 = np.ascontiguousarray(cs.reshape(128, NCH, CH).transpose(1, 0, 2)).astype(np_k)
    sn4 = np.ascontiguousarray(sn.reshape(128, NCH, CH).transpose(1, 0, 2)).astype(np_k)

    jj = np.arange(128)[:, None]
    ii = np.arange(CH)[None, :]
    mask = np.zeros((128, 4 * CH), dtype=np.float64)
    for b in range(4):
        mask[:, b * CH : (b + 1) * CH] = (128 * b + jj) <= ii
    return cs4, sn4, mask


def make_core_inputs(x, Wqkv, Wproj, core):
    """Build the per-core input map (numpy arrays, device dtypes)."""
    b, g = core // 2, core % 2
    np_x = _np_dt(DT_X)
    np_pv = _np_dt(DT_PV)
    np_o = _np_dt(DT_O)

    xT = np.ascontiguousarray(x[b].T)  # [C, T]
    xt4 = np.ascontiguousarray(
        xT.reshape(KT, 128, NCH, CH).transpose(2, 0, 1, 3)
    ).astype(np_x)

    Wq = Wqkv[g * 512 : (g + 1) * 512]
    Wk = Wqkv[C + g * 512 : C + (g + 1) * 512]
    Wv = Wqkv[2 * C + g * 512 : 2 * C + (g + 1) * 512]
    wqkT = np.vstack([Wq, Wk]).T  # [C, 1024]
    wqk = np.ascontiguousarray(wqkT.reshape(KT, 128, 1024)).astype(np_x)
    wvT = Wv.T  # [C, 512]
    wv = np.ascontiguousarray(wvT.reshape(KT, 128, 512)).astype(np_x)
    wpT = Wproj[:, g * 512 : (g + 1) * 512].T  # [512, C]
    wp = np.ascontiguousarray(wpT.reshape(NP, 128, C)).astype(np_o)

    cs4, sn4, mask = _host_tables()
    return {
        "xt4": xt4,
        "wqk": wqk,
        "wv": wv,
        "wp": wp,
        "cs4": cs4,
        "sn4": sn4,
        "mask": mask.astype(np_pv),
    }


LAST_RESULTS = None
_NC_CACHE = None


def kernel(x, Wqkv, Wproj):
    global LAST_RESULTS, _NC_CACHE
    from concourse.bass_utils import run_bass_kernel_spmd

    x = np.asarray(x, dtype=np.float32)
    Wqkv = np.asarray(Wqkv, dtype=np.float32)
    Wproj = np.asarray(Wproj, dtype=np.float32)

    if _NC_CACHE is None:
        _NC_CACHE = build_nc()
    nc = _NC_CACHE
    in_maps = [make_core_inputs(x, Wqkv, Wproj, core) for core in range(NCORES)]
    res = run_bass_kernel_spmd(nc, in_maps, list(range(NCORES)))
    LAST_RESULTS = res

    out = np.empty((B, T, C), dtype=np.float32)
    for b in range(B):
        out[b] = res.results[2 * b]["y"] + res.results[2 * b + 1]["y"]
    return out


# revision 21
# speedup vs baseline: 1.5593x; 1.0163x over previous
"""Causal self-attention (RoPE) Trainium2 Bass kernel.

Problem: B=4, T=2048, C=1024, H=16 heads, D=64, fp32 I/O.
Sharding: 8 cores = 4 (batch) x 2 (head-group TP). Each core computes
qkv/attention/proj for 1 batch and 8 heads, producing a partial
projection output; the host sums the two TP partials per batch.

Per-core pipeline (all chunk-interleaved to overlap PE matmuls with the
ACT-engine exp of the softmax):
  stage A (per 512-token chunk c): qkv projection + RoPE -> q_rot(c)
    (transient), k_rot[:, c] (persistent), v(c) (persistent, with an
    appended ones column for the softmax denominator)
  stage B (per chunk c): for each head, S_T = k^T-layout scores via
    row-packed matmuls, exp on ACT (scale=1/8), causal diag masks on
    DVE, then out_T = v_ext^T @ P_T accumulated over key blocks; row 64
    of the accumulator is the softmax denominator l. Normalize with
    reciprocal + gpsimd partition_broadcast.
  proj (per chunk): y[chunk] = o_T^T @ WprojT, partial over this core's
    512 input features.
"""

import numpy as np
from contextlib import ExitStack

import concourse.bacc as bacc
import concourse.bass as bass
import concourse.mybir as mybir
import concourse.tile as tile

# ---------------- constants ----------------
B = 4
T = 2048
C = 1024
H = 16
D = 64
L = 8  # local heads per core
NCORES = 8
ROPE_BASE = 10000.0

CH = 512  # t-chunk size
NCH = T // CH  # 4 chunks
KT = C // 128  # 8 contraction tiles
NP = L // 2  # 4 head-pair tiles
SCALE = 1.0 / np.sqrt(D)

F32 = mybir.dt.float32
BF16 = mybir.dt.bfloat16

# matmul operand dtypes
DT_X = BF16  # x / Wqkv / Wv operands
DT_K = BF16  # q_rot / k_rot
DT_PV = BF16  # P tiles, v tiles, masks
DT_O = BF16  # o_T tiles / WprojT


def _np_dt(dt):
    return mybir.dt.np(dt)


# ---------------- device kernel ----------------


def attn_body(ctx: ExitStack, tc: tile.TileContext, outs, ins):
    """outs = (y [T, C] f32,); ins = (xt4, wqk, wv, wp, cs4, sn4, mask)."""
    nc = tc.nc
    (y,) = outs if isinstance(outs, (tuple, list)) else (outs,)
    xt4, wqk, wv, wp, cs4, sn4, mask = ins

    TB = T // 128  # 16 key blocks

    consts = ctx.enter_context(tc.tile_pool(name="consts", bufs=1))
    xpool = ctx.enter_context(tc.tile_pool(name="xpool", bufs=16))
    cspool = ctx.enter_context(tc.tile_pool(name="cspool", bufs=4))
    qrpool = ctx.enter_context(tc.tile_pool(name="qrpool", bufs=8))
    rtmp = ctx.enter_context(tc.tile_pool(name="rtmp", bufs=4))
    ptpool = ctx.enter_context(tc.tile_pool(name="ptpool", bufs=6))
    otpool = ctx.enter_context(tc.tile_pool(name="otpool", bufs=8))
    yepool = ctx.enter_context(tc.tile_pool(name="yepool", bufs=3))
    lpool = ctx.enter_context(tc.tile_pool(name="lpool", bufs=3))
    pmisc = ctx.enter_context(tc.tile_pool(name="pmisc", bufs=2, space="PSUM"))
    pss_pool = ctx.enter_context(tc.tile_pool(name="pss", bufs=2, space="PSUM"))
    pso_pool = ctx.enter_context(tc.tile_pool(name="pso", bufs=2, space="PSUM"))

    # persistent tiles
    wqk_sb = [consts.tile([128, 2 * 512], DT_X, name=f"wqk{k}") for k in range(KT)]
    wv_sb = [consts.tile([128, 512], DT_X, name=f"wv{k}") for k in range(KT)]
    wp_sb = [consts.tile([128, C], DT_O, name=f"wp{p}") for p in range(NP)]
    mask_sb = consts.tile([128, 4 * CH], DT_PV, name="masks")
    k_rot = [consts.tile([128, T], DT_K, name=f"krot{p}") for p in range(NP)]
    v_sb = consts.tile([128, TB, L, 65], DT_PV, name="vsb")

    def load_first_chunk():
        # interleave weight and activation loads so the first psum group's
        # operands (wqk[0], xt[0][0]) land first
        xt_sb[0] = []
        for k in range(KT):
            nc.sync.dma_start(wqk_sb[k][:], wqk[k])
            xt = xpool.tile([128, CH], DT_X, name=f"xt0_{k}", tag="xt")
            nc.sync.dma_start(xt[:], xt4[0, k])
            xt_sb[0].append(xt)
        cs_sb[0] = cspool.tile([128, CH], DT_K, name="cs0", tag="cs")
        sn_sb[0] = cspool.tile([128, CH], DT_K, name="sn0", tag="sn")
        nc.sync.dma_start(cs_sb[0][:], cs4[0])
        nc.sync.dma_start(sn_sb[0][:], sn4[0])
        for k in range(KT):
            nc.sync.dma_start(wv_sb[k][:], wv[k])
        # softmax-denominator ones column
        nc.vector.memset(v_sb[:, :, :, 64:65], 1.0)

    def load_consts_late():
        nc.sync.dma_start(mask_sb[:], mask[:])
        for p in range(NP):
            nc.sync.dma_start(wp_sb[p][:], wp[p])

    # per-chunk transient state
    xt_sb = {}
    q_rot = {}
    cs_sb = {}
    sn_sb = {}
    ot_sb = {}

    def load_chunk_inputs(c):
        us = []

        def mk_load(c):
            def f():
                cs_sb[c] = cspool.tile([128, CH], DT_K, name=f"cs{c}", tag="cs")
                sn_sb[c] = cspool.tile([128, CH], DT_K, name=f"sn{c}", tag="sn")
                nc.sync.dma_start(cs_sb[c][:], cs4[c])
                nc.sync.dma_start(sn_sb[c][:], sn4[c])
                xt_sb[c] = []
                for k in range(KT):
                    xt = xpool.tile([128, CH], DT_X, name=f"xt{c}_{k}", tag="xt")
                    nc.sync.dma_start(xt[:], xt4[c, k])
                    xt_sb[c].append(xt)

            return f

        us.append(mk_load(c))
        return us

    def stage_a_units(c):
        """12 units: 8 q/k feature tiles + 4 v t-blocks for chunk c."""
        units = []

        def mk_qk(c, jt):
            def f():
                ps = pmisc.tile([128, CH], F32, name=f"psA{c}_{jt}", tag="pA")
                for k in range(KT):
                    nc.tensor.matmul(
                        ps[:],
                        wqk_sb[k][:, jt * 128 : (jt + 1) * 128],
                        xt_sb[c][k][:],
                        start=(k == 0),
                        stop=(k == KT - 1),
                    )
                # RoPE: rot = raw*cos + swap(raw)*sin_signed, computed in
                # bf16.  PSUM evac on DVE; the cross-partition swap runs as
                # four gpsimd copies (walrus forbids shifted all-SBUF
                # TensorTensor, but plain copies are fine); bf16 muls/adds
                # on DVE get the 4x SBUF mode.
                sn = sn_sb[c]
                q_sb = rtmp.tile([128, CH], DT_K, name=f"qsb{c}_{jt}", tag="qsb")
                nc.vector.tensor_copy(q_sb[:], ps[:])
                qsw = rtmp.tile([128, CH], DT_K, name=f"qsw{c}_{jt}", tag="qsw")
                for blk in range(2):
                    b0 = blk * 64
                    nc.vector.tensor_copy(
                        qsw[b0 : b0 + 32, :], q_sb[b0 + 32 : b0 + 64, :]
                    )
                    nc.vector.tensor_copy(
                        qsw[b0 + 32 : b0 + 64, :], q_sb[b0 : b0 + 32, :]
                    )
                qtmp = rtmp.tile([128, CH], DT_K, name=f"qtmp{c}_{jt}", tag="qtmp")
                nc.vector.tensor_tensor(
                    out=qtmp[:], in0=qsw[:], in1=sn[:], op=mybir.AluOpType.mult
                )
                qraw = rtmp.tile([128, CH], DT_K, name=f"qraw{c}_{jt}", tag="qraw")
                nc.vector.tensor_tensor(
                    out=qraw[:], in0=q_sb[:], in1=cs_sb[c][:],
                    op=mybir.AluOpType.mult,
                )
                if jt < NP:  # q tile
                    dst = qrpool.tile([128, CH], DT_K, name=f"qrot{c}_{jt}", tag="qr")
                    q_rot[(c, jt)] = dst
                    nc.vector.tensor_tensor(
                        out=dst[:], in0=qraw[:], in1=qtmp[:], op=mybir.AluOpType.add
                    )
                else:  # k tile
                    p = jt - NP
                    nc.vector.tensor_tensor(
                        out=k_rot[p][:, c * CH : (c + 1) * CH],
                        in0=qraw[:],
                        in1=qtmp[:],
                        op=mybir.AluOpType.add,
                    )

            return f

        def mk_v(c, tbl):
            tb = c * 4 + tbl

            def f():
                ps = pmisc.tile([128, CH], F32, name=f"psV{c}_{tbl}", tag="pA")
                for k in range(KT):
                    nc.tensor.matmul(
                        ps[:],
                        xt_sb[c][k][:, tbl * 128 : (tbl + 1) * 128],
                        wv_sb[k][:],
                        start=(k == 0),
                        stop=(k == KT - 1),
                    )
                nc.vector.tensor_copy(
                    v_sb[:, tb, :, 0:64],
                    ps[:].rearrange("p (h d) -> p h d", h=L),
                )

            return f

        for jt in range(2 * NP):
            units.append(mk_qk(c, jt))
        for tbl in range(4):
            units.append(mk_v(c, tbl))
        return units

    def stage_b_units(c):
        """per chunk c: 4 head-pairs x (2c+2) key-block-pairs."""
        units = []

        def mk_unit(c, p, jp):
            def f():
                ha, hb = 2 * p, 2 * p + 1
                qt = q_rot[(c, p)]
                kt_ = k_rot[p]
                pss = {}
                for idx, (h, rb) in enumerate(((ha, 0), (hb, 64))):
                    pss[h] = pss_pool.tile(
                        [128, 2 * CH], F32, name=f"pss{c}_{p}_{jp}_{idx}", tag="pss"
                    )
                    for half in range(2):
                        jb = 2 * jp + half
                        nc.tensor.matmul(
                            pss[h][:, half * CH : (half + 1) * CH],
                            kt_[rb : rb + 64, jb * 128 : (jb + 1) * 128],
                            qt[rb : rb + 64, :],
                            start=True,
                            stop=True,
                        )
                for idx, h in enumerate((ha, hb)):
                    pt = ptpool.tile(
                        [128, 2 * CH], DT_PV, name=f"pt{c}_{p}_{jp}_{idx}", tag="pt"
                    )
                    nc.scalar.activation(
                        pt[:],
                        pss[h][:],
                        mybir.ActivationFunctionType.Exp,
                        scale=float(SCALE),
                    )
                    if jp >= 2 * c:  # diagonal pair -> causal mask
                        moff = (jp - 2 * c) * 2 * CH
                        nc.vector.tensor_tensor(
                            out=pt[:],
                            in0=pt[:],
                            in1=mask_sb[:, moff : moff + 2 * CH],
                            op=mybir.AluOpType.mult,
                        )
                    pso = pso_unit[(c, h)]
                    for half in range(2):
                        jb = 2 * jp + half
                        nc.tensor.matmul(
                            pso[:],
                            v_sb[:, jb, h, 0:65],
                            pt[:, half * CH : (half + 1) * CH],
                            start=(jp == 0 and half == 0),
                            stop=(jp == 2 * c + 1 and half == 1),
                        )

            return f

        def mk_alloc_pso(c, p):
            def f():
                for h in (2 * p, 2 * p + 1):
                    pso_unit[(c, h)] = pso_pool.tile(
                        [65, CH], F32, name=f"pso{c}_{h}", tag="pso"
                    )

            return f

        def mk_norm(c, p):
            def f():
                ot = ot_sb[(c, p)] = otpool.tile(
                    [128, CH], DT_O, name=f"ot{c}_{p}", tag="ot"
                )
                for idx, h in enumerate((2 * p, 2 * p + 1)):
                    pso = pso_unit[(c, h)]
                    lsb = lpool.tile([1, CH], F32, name=f"lsb{c}_{h}", tag="lsb")
                    nc.vector.tensor_copy(lsb[:], pso[64:65, :])
                    linv = lpool.tile([1, CH], F32, name=f"linv{c}_{h}", tag="linv")
                    nc.vector.reciprocal_approx_fast(linv[:], lsb[:])
                    lb = lpool.tile([64, CH], F32, name=f"lb{c}_{h}", tag="lb")
                    nc.gpsimd.partition_broadcast(lb[:], linv[:])
                    nc.vector.tensor_tensor(
                        out=ot[idx * 64 : (idx + 1) * 64, :],
                        in0=pso[0:64, :],
                        in1=lb[:],
                        op=mybir.AluOpType.mult,
                    )

            return f

        pso_unit = {}
        for p in range(NP):
            units.append(mk_alloc_pso(c, p))
            for jp in range(2 * c + 2):
                units.append(mk_unit(c, p, jp))
            units.append(mk_norm(c, p))
        return units

    def proj_units(c):
        units = []

        def mk_proj(c, tbl, oc):
            def f():
                ps = pmisc.tile([128, CH], F32, name=f"psY{c}_{tbl}_{oc}", tag="pA")
                for p in range(NP):
                    nc.tensor.matmul(
                        ps[:],
                        ot_sb[(c, p)][:, tbl * 128 : (tbl + 1) * 128],
                        wp_sb[p][:, oc * CH : (oc + 1) * CH],
                        start=(p == 0),
                        stop=(p == NP - 1),
                    )
                ye = yepool.tile([128, CH], F32, name=f"ye{c}_{tbl}_{oc}", tag="ye")
                nc.vector.tensor_copy(ye[:], ps[:])
                nc.sync.dma_start(
                    y[c * CH + tbl * 128 : c * CH + (tbl + 1) * 128,
                      oc * CH : (oc + 1) * CH],
                    ye[:],
                )

            return f

        for tbl in range(4):
            for oc in range(C // CH):
                units.append(mk_proj(c, tbl, oc))
        return units

    def emit_interleaved(primary, secondary):
        """Emit primary units with secondary units spread between them."""
        if not primary:
            for u in secondary:
                u()
            return
        ns, npri = len(secondary), len(primary)
        si = 0
        for i, u in enumerate(primary):
            u()
            want = (i + 1) * ns // npri
            while si < want:
                secondary[si]()
                si += 1

    # ---- emission ----
    load_first_chunk()
    for u in stage_a_units(0):
        u()
    load_consts_late()
    for c in range(NCH):
        fill = []
        if c + 1 < NCH:
            fill += load_chunk_inputs(c + 1)
            fill += stage_a_units(c + 1)
        if c > 0:
            fill += proj_units(c - 1)
        emit_interleaved(stage_b_units(c), fill)
    for u in proj_units(NCH - 1):
        u()


def build_nc():
    nc = bacc.Bacc("TRN2", target_bir_lowering=False, debug=False)
    xt4 = nc.declare_dram_parameter("xt4", [NCH, KT, 128, CH], DT_X, isOutput=False)
    wqk = nc.declare_dram_parameter("wqk", [KT, 128, 1024], DT_X, isOutput=False)
    wv = nc.declare_dram_parameter("wv", [KT, 128, 512], DT_X, isOutput=False)
    wp = nc.declare_dram_parameter("wp", [NP, 128, C], DT_O, isOutput=False)
    cs4 = nc.declare_dram_parameter("cs4", [NCH, 128, CH], DT_K, isOutput=False)
    sn4 = nc.declare_dram_parameter("sn4", [NCH, 128, CH], DT_K, isOutput=False)
    mask = nc.declare_dram_parameter("mask", [128, 4 * CH], DT_PV, isOutput=False)
    yout = nc.declare_dram_parameter("y", [T, C], F32, isOutput=True)

    with tile.TileContext(nc) as tc:
        with ExitStack() as ctx:
            attn_body(
                ctx, tc, (yout[:],),
                (xt4[:], wqk[:], wv[:], wp[:], cs4[:], sn4[:], mask[:]),
            )
    nc.compile()
    return nc


# ---------------- host side ----------------


def _rope_tables_np():
    inv_freq = 1.0 / (ROPE_BASE ** (np.arange(0, D, 2, dtype=np.float64) / D))
    t = np.arange(T, dtype=np.float64)
    freqs = np.outer(t, inv_freq)  # [T, 32]
    emb = np.concatenate([freqs, freqs], axis=-1)  # [T, 64]
    return np.cos(emb), np.sin(emb)  # [T, 64] each


def _host_tables():
    cos, sin = _rope_tables_np()  # [T, 64]
    d_of_r = np.arange(128) % 64
    cs = cos[:, d_of_r].T.astype(np.float32)  # [128, T]
    sn_abs = sin[:, d_of_r].T
    sign = np.where((d_of_r % 64) < 32, -1.0, 1.0)[:, None]
    sn = (sn_abs * sign).astype(np.float32)  # [128, T]
    np_k = _np_dt(DT_K)
    cs4 = np.ascontiguousarray(cs.reshape(128, NCH, CH).transpose(1, 0, 2)).astype(np_k)
    sn4 = np.ascontiguousarray(sn.reshape(128, NCH, CH).transpose(1, 0, 2)).astype(np_k)

    jj = np.arange(128)[:, None]
    ii = np.arange(CH)[None, :]
    mask = np.zeros((128, 4 * CH), dtype=np.float64)
    for b in range(4):
        mask[:, b * CH : (b + 1) * CH] = (128 * b + jj) <= ii
    return cs4, sn4, mask


def make_core_inputs(x, Wqkv, Wproj, core):
    """Build the per-core input map (numpy arrays, device dtypes)."""
    b, g = core // 2, core % 2
    np_x = _np_dt(DT_X)
    np_pv = _np_dt(DT_PV)
    np_o = _np_dt(DT_O)

    xT = np.ascontiguousarray(x[b].T)  # [C, T]
    xt4 = np.ascontiguousarray(
        xT.reshape(KT, 128, NCH, CH).transpose(2, 0, 1, 3)
    ).astype(np_x)

    Wq = Wqkv[g * 512 : (g + 1) * 512]
    Wk = Wqkv[C + g * 512 : C + (g + 1) * 512]
    Wv = Wqkv[2 * C + g * 512 : 2 * C + (g + 1) * 512]
    wqkT = np.vstack([Wq, Wk]).T  # [C, 1024]
    wqk = np.ascontiguousarray(wqkT.reshape(KT, 128, 1024)).astype(np_x)
    wvT = Wv.T  # [C, 512]
    wv = np.ascontiguousarray(wvT.reshape(KT, 128, 512)).astype(np_x)
    wpT = Wproj[:, g * 512 : (g + 1) * 512].T  # [512, C]
    wp = np.ascontiguousarray(wpT.reshape(NP, 128, C)).astype(np_o)

    cs4, sn4, mask = _host_tables()
    return {
        "xt4": xt4,
        "wqk": wqk,
        "wv": wv,
        "wp": wp,
        "cs4": cs4,
        "sn4": sn4,
        "mask": mask.astype(np_pv),
    }


LAST_RESULTS = None
_NC_CACHE = None


def kernel(x, Wqkv, Wproj):
    global LAST_RESULTS, _NC_CACHE
    from concourse.bass_utils import run_bass_kernel_spmd

    x = np.asarray(x, dtype=np.float32)
    Wqkv = np.asarray(Wqkv, dtype=np.float32)
    Wproj = np.asarray(Wproj, dtype=np.float32)

    if _NC_CACHE is None:
        _NC_CACHE = build_nc()
    nc = _NC_CACHE
    in_maps = [make_core_inputs(x, Wqkv, Wproj, core) for core in range(NCORES)]
    res = run_bass_kernel_spmd(nc, in_maps, list(range(NCORES)))
    LAST_RESULTS = res

    out = np.empty((B, T, C), dtype=np.float32)
    for b in range(B):
        out[b] = res.results[2 * b]["y"] + res.results[2 * b + 1]["y"]
    return out
